# revision 2
# baseline (speedup 1.0000x reference)
"""Trainium2 Bass kernel for nn_Bessel: out = i0e(z) * exp(z - 2a), z = 2a*sqrt((1+x@yT)/2), a=10.

Math: out = exp(z - 20 + ln i0e(z)) = exp(A + B*z + C*z^2 - 20) via a minimax
fit of t(z) = z + ln i0e(z) over z in [7.30, 19.20] (max rel err ~5.0e-3).

Key identity used by mode "beta2": the quadratic correction needs no extra
PSUM read since  B*z + C*z^2 = C * v  with  v = (z + B/C) * z  computable on
DVE from the SBUF z-tile alone:
  PE:  c = x@yT into PSUM  (bf16x2 split: c = [xh;xl]@[yh;yh] + xh@yl)
  ACT: z = Sqrt(200c+200) evacuating PSUM   (sole PSUM reader -> PE/ACT
       pipeline runs at full ACT rate; DVE off the recycle path)
  DVE: v = (z + B/C) * z  in-place on the z tile (SBUF-only stt)
  ACT: obf = Exp(C*v + (A-20)) -> bf16
  DMA: obf tile -> HBM (bf16 halves write traffic; host upcasts to f32)

ACT-table batching: M-tiles are processed in GROUPs of 4 (all Sqrt evacs,
then all Exps) so each iteration pays 4 table-set loads; order is pinned
with add_dep_helper so walrus cannot interleave sqrt/exp across a group
boundary.

Mode "beta" is the previous structure (stt reads PSUM, f32 out) kept for A/B.
"""

import contextlib

import numpy as np

import concourse.bacc as bacc
import concourse.mybir as mybir
from concourse.tile import TileContext
from concourse.tile_autobufs import add_dep_helper
from concourse.bass_utils import run_bass_kernel_spmd

AF = mybir.ActivationFunctionType
OP = mybir.AluOpType
F32 = mybir.dt.float32
BF16 = mybir.dt.bfloat16
BFNP = mybir.dt.np(BF16)

N_CORES = 8
N_ROWS, M_COLS, DIM = 8192, 8192, 64
ROWS = N_ROWS // N_CORES          # 1024 rows of x per core
MTILES = ROWS // 128              # 8 partition tiles per core

# minimax coefficients for t(z) = z + ln(i0e(z)) on z in [7.30, 19.20]
BETA_A, BETA_B, BETA_C = -1.36067207867, 0.913667220475, 0.00171853078443

MODE = "beta2"

_cache = {}


def _build2(group=4, psum_fd=2048, obf_bufs=2, zw_extra=0, exp_split=1,
            iters=1, out_bf16=True):
    """mode beta2: PSUM-decoupled DVE correction + bf16 output."""
    nc = bacc.Bacc(None, target_bir_lowering=False)
    xs_d = nc.dram_tensor("xs", [2 * DIM, ROWS], BF16, kind="ExternalInput")
    ys_d = nc.dram_tensor("ys", [2 * DIM, M_COLS], BF16, kind="ExternalInput")
    yl_d = nc.dram_tensor("yl", [DIM, M_COLS], BF16, kind="ExternalInput")
    out_d = nc.dram_tensor("out", [ROWS, M_COLS], BF16 if out_bf16 else F32,
                           kind="ExternalOutput")

    stt_scalar = BETA_B / BETA_C                   # v = (z + B/C) * z
    exp_scale = BETA_C                             # exp(C*v + A - 20)
    exp_bias = BETA_A - 20.0

    with TileContext(nc) as tc:
        with (
            tc.tile_pool(name="inp", bufs=1) as inp,
            tc.tile_pool(name="consts", bufs=1) as consts,
            tc.tile_pool(name="zw", bufs=group + zw_extra) as zwpool,
            tc.tile_pool(name="obf", bufs=obf_bufs) as obfpool,
            tc.tile_pool(name="psum", bufs=4096 // psum_fd, space="PSUM") as psum,
        ):
            xs = inp.tile([2 * DIM, ROWS], BF16)
            ys = inp.tile([2 * DIM, M_COLS], BF16)
            yl = inp.tile([DIM, M_COLS], BF16)
            nc.sync.dma_start(out=xs[:], in_=xs_d[:])
            for q in range(0, M_COLS, 2048):
                nc.sync.dma_start(out=ys[:, q:q + 2048], in_=ys_d[:, q:q + 2048])
                nc.sync.dma_start(out=yl[:, q:q + 2048], in_=yl_d[:, q:q + 2048])

            b200 = consts.tile([128, 1], F32)
            nc.gpsimd.memset(b200[:], 200.0)
            bexp = consts.tile([128, 1], F32)
            nc.gpsimd.memset(bexp[:], float(exp_bias))

            nchunk = M_COLS // psum_fd
            mtile_groups = [
                list(range(g, min(g + group, MTILES)))
                for g in range(0, MTILES, group)
            ]
            loop_cm = tc.For_i(0, iters) if iters > 1 else contextlib.nullcontext(0)
            with loop_cm as _i:
                last_exp = None
                for grp in mtile_groups:
                    zw_tiles = {}
                    last_evac = None
                    for m in grp:
                        zw = zwpool.tile([128, M_COLS], F32, tag="zw")
                        zw_tiles[m] = zw
                        msl = slice(m * 128, (m + 1) * 128)
                        for nb in range(nchunk):
                            pt = psum.tile([128, psum_fd], F32, tag="ps")
                            # all hi-part matmuls first, then lo: two
                            # stationary loads per chunk instead of eight
                            for j in range(psum_fd // 512):
                                col = nb * psum_fd + j * 512
                                nc.tensor.matmul(
                                    pt[:, j * 512:(j + 1) * 512],
                                    xs[:, msl], ys[:, col:col + 512],
                                    start=True, stop=False,
                                )
                            for j in range(psum_fd // 512):
                                col = nb * psum_fd + j * 512
                                nc.tensor.matmul(
                                    pt[:, j * 512:(j + 1) * 512],
                                    xs[:DIM, msl], yl[:, col:col + 512],
                                    start=False, stop=True,
                                )
                            sl = slice(nb * psum_fd, (nb + 1) * psum_fd)
                            ev = nc.scalar.activation(
                                zw[:, sl], pt[:], AF.Sqrt, bias=b200[:], scale=200.0
                            )
                            if last_exp is not None:
                                # keep this group's sqrts after the previous
                                # group's exps (table-set batching)
                                add_dep_helper(
                                    ev.ins, last_exp.ins, sync=False,
                                    reason="batch sqrt after prev group exp",
                                )
                            last_evac = ev
                            # v = (z + B/C) * z  -- SBUF-only, PSUM already free
                            nc.vector.scalar_tensor_tensor(
                                zw[:, sl], zw[:, sl], stt_scalar, zw[:, sl],
                                OP.add, OP.mult,
                            )
                    for m in grp:
                        zw = zw_tiles[m]
                        efd = M_COLS // exp_split
                        obf = obfpool.tile([128, M_COLS], BF16 if out_bf16 else F32,
                                           tag="obf")
                        for e in range(exp_split):
                            esl = slice(e * efd, (e + 1) * efd)
                            exp_inst = nc.scalar.activation(
                                obf[:, esl], zw[:, esl], AF.Exp,
                                bias=bexp[:], scale=float(exp_scale)
                            )
                            add_dep_helper(
                                exp_inst.ins, last_evac.ins, sync=False,
                                reason="batch exp after group sqrt (table switch)",
                            )
                            last_exp = exp_inst
                            nc.sync.dma_start(
                                out=out_d[m * 128:(m + 1) * 128, esl],
                                in_=obf[:, esl],
                            )

    nc.finalize()
    return nc


def _build_beta(group=3, zw_bufs=None, exp_split=2, psum_fd=1024,
                iters=1, out_bf16=False, obf_bufs=2):
    """previous structure: stt reads PSUM, default f32 out (for A/B)."""
    nc = bacc.Bacc(None, target_bir_lowering=False)
    xs_d = nc.dram_tensor("xs", [2 * DIM, ROWS], BF16, kind="ExternalInput")
    ys_d = nc.dram_tensor("ys", [2 * DIM, M_COLS], BF16, kind="ExternalInput")
    yl_d = nc.dram_tensor("yl", [DIM, M_COLS], BF16, kind="ExternalInput")
    out_d = nc.dram_tensor("out", [ROWS, M_COLS], BF16 if out_bf16 else F32,
                           kind="ExternalOutput")

    stt_scalar = 200.0 * BETA_C / BETA_B
    exp_scale = BETA_B
    exp_bias = BETA_A + 200.0 * BETA_C - 20.0

    with TileContext(nc) as tc:
        with (
            tc.tile_pool(name="inp", bufs=1) as inp,
            tc.tile_pool(name="consts", bufs=1) as consts,
            tc.tile_pool(name="zw", bufs=zw_bufs or (group if out_bf16 else group + 1)) as zwpool,
            tc.tile_pool(name="obf", bufs=obf_bufs) as obfpool,
            tc.tile_pool(name="psum", bufs=4096 // psum_fd, space="PSUM") as psum,
        ):
            xs = inp.tile([2 * DIM, ROWS], BF16)
            ys = inp.tile([2 * DIM, M_COLS], BF16)
            yl = inp.tile([DIM, M_COLS], BF16)
            nc.sync.dma_start(out=xs[:], in_=xs_d[:])
            for q in range(0, M_COLS, 2048):
                nc.sync.dma_start(out=ys[:, q:q + 2048], in_=ys_d[:, q:q + 2048])
                nc.sync.dma_start(out=yl[:, q:q + 2048], in_=yl_d[:, q:q + 2048])

            b200 = consts.tile([128, 1], F32)
            nc.gpsimd.memset(b200[:], 200.0)
            bexp = consts.tile([128, 1], F32)
            nc.gpsimd.memset(bexp[:], float(exp_bias))

            nchunk = M_COLS // psum_fd
            mtile_groups = [
                list(range(g, min(g + group, MTILES)))
                for g in range(0, MTILES, group)
            ]
            loop_cm = tc.For_i(0, iters) if iters > 1 else contextlib.nullcontext(0)
            with loop_cm as _i:
              for grp in mtile_groups:
                  zw_tiles = {}
                  last_evac = None
                  for m in grp:
                      zw = zwpool.tile([128, M_COLS], F32, tag="zw")
                      zw_tiles[m] = zw
                      msl = slice(m * 128, (m + 1) * 128)
                      for nb in range(nchunk):
                          pt = psum.tile([128, psum_fd], F32, tag="ps")
                          for j in range(psum_fd // 512):
                              col = nb * psum_fd + j * 512
                              csl = slice(col, col + 512)
                              nc.tensor.matmul(
                                  pt[:, j * 512:(j + 1) * 512],
                                  xs[:, msl], ys[:, csl],
                                  start=True, stop=False,
                              )
                              nc.tensor.matmul(
                                  pt[:, j * 512:(j + 1) * 512],
                                  xs[:DIM, msl], yl[:, csl],
                                  start=False, stop=True,
                              )
                          sl = slice(nb * psum_fd, (nb + 1) * psum_fd)
                          last_evac = nc.scalar.activation(
                              zw[:, sl], pt[:], AF.Sqrt, bias=b200[:], scale=200.0
                          )
                          nc.vector.scalar_tensor_tensor(
                              zw[:, sl], pt[:], stt_scalar, zw[:, sl],
                              OP.mult, OP.add,
                          )
                  for m in grp:
                      zw = zw_tiles[m]
                      efd = M_COLS // exp_split
                      if out_bf16:
                          obf = obfpool.tile([128, M_COLS], BF16, tag="obf")
                      for e in range(exp_split):
                          esl = slice(e * efd, (e + 1) * efd)
                          etgt = obf[:, esl] if out_bf16 else zw[:, esl]
                          exp_inst = nc.scalar.activation(
                              etgt, zw[:, esl], AF.Exp,
                              bias=bexp[:], scale=float(exp_scale)
                          )
                          if last_evac is not None:
                              add_dep_helper(
                                  exp_inst.ins, last_evac.ins, sync=False,
                                  reason="batch exp after group sqrt (table switch)",
                              )
                          nc.sync.dma_start(
                              out=out_d[m * 128:(m + 1) * 128, esl], in_=etgt
                          )

    nc.finalize()
    return nc


def _build(mode, iters=1, **kw):
    if mode == "beta2":
        return _build2(iters=iters, **kw)
    return _build_beta(iters=iters, **kw)


LAST_RESULTS = None


def _split_bf16(a):
    hi = a.astype(BFNP)
    lo = (a - hi.astype(np.float32)).astype(BFNP)
    return hi, lo


def kernel(x: np.ndarray, y: np.ndarray) -> np.ndarray:
    global LAST_RESULTS
    x = np.ascontiguousarray(x, dtype=np.float32)
    y = np.ascontiguousarray(y, dtype=np.float32)
    assert x.shape == (N_ROWS, DIM) and y.shape == (M_COLS, DIM)

    if MODE not in _cache:
        _cache[MODE] = _build(MODE)
    nc = _cache[MODE]

    yT = y.T
    yh, yl = _split_bf16(yT)
    ys = np.ascontiguousarray(np.concatenate([yh, yh], axis=0))
    yl = np.ascontiguousarray(yl)

    in_maps = []
    for i in range(N_CORES):
        xT = x[i * ROWS:(i + 1) * ROWS].T
        xh, xl = _split_bf16(xT)
        xstack = np.ascontiguousarray(np.concatenate([xh, xl], axis=0))
        in_maps.append({"xs": xstack, "ys": ys, "yl": yl})

    LAST_RESULTS = run_bass_kernel_spmd(nc, in_maps, list(range(N_CORES)))
    out = np.concatenate([r["out"] for r in LAST_RESULTS.results], axis=0)
    if out.dtype != np.float32:
        out = out.astype(np.float32)
    return out


# revision 4
# speedup vs baseline: 1.0246x; 1.0246x over previous
"""Trainium2 Bass kernel for nn_Bessel: out = i0e(z) * exp(z - 2a), z = 2a*sqrt((1+x@yT)/2), a=10.

Math: out = exp(z - 20 + ln i0e(z)) = exp(A + B*z + C*z^2 - 20) via a minimax
fit of t(z) = z + ln i0e(z) over z in [7.30, 19.20] (max rel err ~5.0e-3).

Key identity used by mode "beta2": the quadratic correction needs no extra
PSUM read since  B*z + C*z^2 = C * v  with  v = (z + B/C) * z  computable on
DVE from the SBUF z-tile alone:
  PE:  c = x@yT into PSUM  (bf16x2 split: c = [xh;xl]@[yh;yh] + xh@yl)
  ACT: z = Sqrt(200c+200) evacuating PSUM   (sole PSUM reader -> PE/ACT
       pipeline runs at full ACT rate; DVE off the recycle path)
  DVE: v = (z + B/C) * z  in-place on the z tile (SBUF-only stt)
  ACT: obf = Exp(C*v + (A-20)) -> bf16
  DMA: obf tile -> HBM (bf16 halves write traffic; host upcasts to f32)

ACT-table batching: M-tiles are processed in GROUPs of 4 (all Sqrt evacs,
then all Exps) so each iteration pays 4 table-set loads; order is pinned
with add_dep_helper so walrus cannot interleave sqrt/exp across a group
boundary.

Mode "beta" is the previous structure (stt reads PSUM, f32 out) kept for A/B.
"""

import contextlib

import numpy as np

import concourse.bacc as bacc
import concourse.mybir as mybir
from concourse.tile import TileContext
from concourse.tile_autobufs import add_dep_helper
from concourse.bass_utils import run_bass_kernel_spmd

AF = mybir.ActivationFunctionType
OP = mybir.AluOpType
F32 = mybir.dt.float32
BF16 = mybir.dt.bfloat16
BFNP = mybir.dt.np(BF16)

N_CORES = 8
N_ROWS, M_COLS, DIM = 8192, 8192, 64
ROWS = N_ROWS // N_CORES          # 1024 rows of x per core
MTILES = ROWS // 128              # 8 partition tiles per core

# minimax coefficients for t(z) = z + ln(i0e(z)) on z in [7.30, 19.20]
BETA_A, BETA_B, BETA_C = -1.36067207867, 0.913667220475, 0.00171853078443

# mode "gamma": minimax fit of the whole exponent as p + r*sqrt(a*u + b),
# u = 200c+200 in [50, 368]; max abs err on the exponent 2.12e-3.
# sqrt input affine = (200a)*c + (200a+b); exp input affine = r*z + p.
GAM_P = -22.179313758272478
GAM_R = 0.7814668006400919
GAM_SQ_SCALE = 314.6476142409728          # 200*a
GAM_SQ_BIAS = 325.04327804569425          # 200*a + b

MODE = "gamma"

_cache = {}


def _build_gamma(group=4, psum_fd=2048, obf_bufs=2, zw_extra=0, exp_split=1,
                 iters=1, zw_fp16=False):
    """mode gamma: exponent = p + r*sqrt(a*u+b) -> no DVE pass at all.

    PE -> ACT Sqrt (PSUM evac, input affine does everything) -> ACT Exp ->
    bf16 out DMA.  M-tiles processed in GROUPs per ACT table set.
    """
    nc = bacc.Bacc(None, target_bir_lowering=False)
    xs_d = nc.dram_tensor("xs", [2 * DIM, ROWS], BF16, kind="ExternalInput")
    ys_d = nc.dram_tensor("ys", [2 * DIM, M_COLS], BF16, kind="ExternalInput")
    yl_d = nc.dram_tensor("yl", [DIM, M_COLS], BF16, kind="ExternalInput")
    out_d = nc.dram_tensor("out", [ROWS, M_COLS], BF16, kind="ExternalOutput")

    zw_dt = mybir.dt.float16 if zw_fp16 else F32

    with TileContext(nc) as tc:
        with (
            tc.tile_pool(name="inp", bufs=1) as inp,
            tc.tile_pool(name="consts", bufs=1) as consts,
            tc.tile_pool(name="zw", bufs=group + zw_extra) as zwpool,
            tc.tile_pool(name="obf", bufs=obf_bufs) as obfpool,
            tc.tile_pool(name="psum", bufs=4096 // psum_fd, space="PSUM") as psum,
        ):
            xs = inp.tile([2 * DIM, ROWS], BF16)
            ys = inp.tile([2 * DIM, M_COLS], BF16)
            yl = inp.tile([DIM, M_COLS], BF16)
            nc.sync.dma_start(out=xs[:], in_=xs_d[:])
            for q in range(0, M_COLS, 2048):
                nc.sync.dma_start(out=ys[:, q:q + 2048], in_=ys_d[:, q:q + 2048])
                nc.sync.dma_start(out=yl[:, q:q + 2048], in_=yl_d[:, q:q + 2048])

            bsq = consts.tile([128, 1], F32)
            nc.gpsimd.memset(bsq[:], float(GAM_SQ_BIAS))
            bexp = consts.tile([128, 1], F32)
            nc.gpsimd.memset(bexp[:], float(GAM_P))

            nchunk = M_COLS // psum_fd
            mtile_groups = [
                list(range(g, min(g + group, MTILES)))
                for g in range(0, MTILES, group)
            ]
            loop_cm = tc.For_i(0, iters) if iters > 1 else contextlib.nullcontext(0)
            with loop_cm as _i:
                last_exp = None
                for grp in mtile_groups:
                    zw_tiles = {}
                    last_evac = None
                    for m in grp:
                        zw = zwpool.tile([128, M_COLS], zw_dt, tag="zw")
                        zw_tiles[m] = zw
                        msl = slice(m * 128, (m + 1) * 128)
                        for nb in range(nchunk):
                            pt = psum.tile([128, psum_fd], F32, tag="ps")
                            for j in range(psum_fd // 512):
                                col = nb * psum_fd + j * 512
                                nc.tensor.matmul(
                                    pt[:, j * 512:(j + 1) * 512],
                                    xs[:, msl], ys[:, col:col + 512],
                                    start=True, stop=False,
                                )
                            for j in range(psum_fd // 512):
                                col = nb * psum_fd + j * 512
                                nc.tensor.matmul(
                                    pt[:, j * 512:(j + 1) * 512],
                                    xs[:DIM, msl], yl[:, col:col + 512],
                                    start=False, stop=True,
                                )
                            sl = slice(nb * psum_fd, (nb + 1) * psum_fd)
                            ev = nc.scalar.activation(
                                zw[:, sl], pt[:], AF.Sqrt,
                                bias=bsq[:], scale=float(GAM_SQ_SCALE)
                            )
                            if last_exp is not None:
                                add_dep_helper(
                                    ev.ins, last_exp.ins, sync=False,
                                    reason="batch sqrt after prev group exp",
                                )
                            last_evac = ev
                    for m in grp:
                        zw = zw_tiles[m]
                        efd = M_COLS // exp_split
                        obf = obfpool.tile([128, M_COLS], BF16, tag="obf")
                        for e in range(exp_split):
                            esl = slice(e * efd, (e + 1) * efd)
                            exp_inst = nc.scalar.activation(
                                obf[:, esl], zw[:, esl], AF.Exp,
                                bias=bexp[:], scale=float(GAM_R)
                            )
                            add_dep_helper(
                                exp_inst.ins, last_evac.ins, sync=False,
                                reason="batch exp after group sqrt (table switch)",
                            )
                            last_exp = exp_inst
                            nc.sync.dma_start(
                                out=out_d[m * 128:(m + 1) * 128, esl],
                                in_=obf[:, esl],
                            )

    nc.finalize()
    return nc


def _build2(group=4, psum_fd=2048, obf_bufs=2, zw_extra=0, exp_split=1,
            iters=1, out_bf16=True):
    """mode beta2: PSUM-decoupled DVE correction + bf16 output."""
    nc = bacc.Bacc(None, target_bir_lowering=False)
    xs_d = nc.dram_tensor("xs", [2 * DIM, ROWS], BF16, kind="ExternalInput")
    ys_d = nc.dram_tensor("ys", [2 * DIM, M_COLS], BF16, kind="ExternalInput")
    yl_d = nc.dram_tensor("yl", [DIM, M_COLS], BF16, kind="ExternalInput")
    out_d = nc.dram_tensor("out", [ROWS, M_COLS], BF16 if out_bf16 else F32,
                           kind="ExternalOutput")

    stt_scalar = BETA_B / BETA_C                   # v = (z + B/C) * z
    exp_scale = BETA_C                             # exp(C*v + A - 20)
    exp_bias = BETA_A - 20.0

    with TileContext(nc) as tc:
        with (
            tc.tile_pool(name="inp", bufs=1) as inp,
            tc.tile_pool(name="consts", bufs=1) as consts,
            tc.tile_pool(name="zw", bufs=group + zw_extra) as zwpool,
            tc.tile_pool(name="obf", bufs=obf_bufs) as obfpool,
            tc.tile_pool(name="psum", bufs=4096 // psum_fd, space="PSUM") as psum,
        ):
            xs = inp.tile([2 * DIM, ROWS], BF16)
            ys = inp.tile([2 * DIM, M_COLS], BF16)
            yl = inp.tile([DIM, M_COLS], BF16)
            nc.sync.dma_start(out=xs[:], in_=xs_d[:])
            for q in range(0, M_COLS, 2048):
                nc.sync.dma_start(out=ys[:, q:q + 2048], in_=ys_d[:, q:q + 2048])
                nc.sync.dma_start(out=yl[:, q:q + 2048], in_=yl_d[:, q:q + 2048])

            b200 = consts.tile([128, 1], F32)
            nc.gpsimd.memset(b200[:], 200.0)
            bexp = consts.tile([128, 1], F32)
            nc.gpsimd.memset(bexp[:], float(exp_bias))

            nchunk = M_COLS // psum_fd
            mtile_groups = [
                list(range(g, min(g + group, MTILES)))
                for g in range(0, MTILES, group)
            ]
            loop_cm = tc.For_i(0, iters) if iters > 1 else contextlib.nullcontext(0)
            with loop_cm as _i:
                last_exp = None
                for grp in mtile_groups:
                    zw_tiles = {}
                    last_evac = None
                    for m in grp:
                        zw = zwpool.tile([128, M_COLS], F32, tag="zw")
                        zw_tiles[m] = zw
                        msl = slice(m * 128, (m + 1) * 128)
                        for nb in range(nchunk):
                            pt = psum.tile([128, psum_fd], F32, tag="ps")
                            # all hi-part matmuls first, then lo: two
                            # stationary loads per chunk instead of eight
                            for j in range(psum_fd // 512):
                                col = nb * psum_fd + j * 512
                                nc.tensor.matmul(
                                    pt[:, j * 512:(j + 1) * 512],
                                    xs[:, msl], ys[:, col:col + 512],
                                    start=True, stop=False,
                                )
                            for j in range(psum_fd // 512):
                                col = nb * psum_fd + j * 512
                                nc.tensor.matmul(
                                    pt[:, j * 512:(j + 1) * 512],
                                    xs[:DIM, msl], yl[:, col:col + 512],
                                    start=False, stop=True,
                                )
                            sl = slice(nb * psum_fd, (nb + 1) * psum_fd)
                            ev = nc.scalar.activation(
                                zw[:, sl], pt[:], AF.Sqrt, bias=b200[:], scale=200.0
                            )
                            if last_exp is not None:
                                # keep this group's sqrts after the previous
                                # group's exps (table-set batching)
                                add_dep_helper(
                                    ev.ins, last_exp.ins, sync=False,
                                    reason="batch sqrt after prev group exp",
                                )
                            last_evac = ev
                            # v = (z + B/C) * z  -- SBUF-only, PSUM already free
                            nc.vector.scalar_tensor_tensor(
                                zw[:, sl], zw[:, sl], stt_scalar, zw[:, sl],
                                OP.add, OP.mult,
                            )
                    for m in grp:
                        zw = zw_tiles[m]
                        efd = M_COLS // exp_split
                        obf = obfpool.tile([128, M_COLS], BF16 if out_bf16 else F32,
                                           tag="obf")
                        for e in range(exp_split):
                            esl = slice(e * efd, (e + 1) * efd)
                            exp_inst = nc.scalar.activation(
                                obf[:, esl], zw[:, esl], AF.Exp,
                                bias=bexp[:], scale=float(exp_scale)
                            )
                            add_dep_helper(
                                exp_inst.ins, last_evac.ins, sync=False,
                                reason="batch exp after group sqrt (table switch)",
                            )
                            last_exp = exp_inst
                            nc.sync.dma_start(
                                out=out_d[m * 128:(m + 1) * 128, esl],
                                in_=obf[:, esl],
                            )

    nc.finalize()
    return nc


def _build_beta(group=3, zw_bufs=None, exp_split=2, psum_fd=1024,
                iters=1, out_bf16=False, obf_bufs=2):
    """previous structure: stt reads PSUM, default f32 out (for A/B)."""
    nc = bacc.Bacc(None, target_bir_lowering=False)
    xs_d = nc.dram_tensor("xs", [2 * DIM, ROWS], BF16, kind="ExternalInput")
    ys_d = nc.dram_tensor("ys", [2 * DIM, M_COLS], BF16, kind="ExternalInput")
    yl_d = nc.dram_tensor("yl", [DIM, M_COLS], BF16, kind="ExternalInput")
    out_d = nc.dram_tensor("out", [ROWS, M_COLS], BF16 if out_bf16 else F32,
                           kind="ExternalOutput")

    stt_scalar = 200.0 * BETA_C / BETA_B
    exp_scale = BETA_B
    exp_bias = BETA_A + 200.0 * BETA_C - 20.0

    with TileContext(nc) as tc:
        with (
            tc.tile_pool(name="inp", bufs=1) as inp,
            tc.tile_pool(name="consts", bufs=1) as consts,
            tc.tile_pool(name="zw", bufs=zw_bufs or (group if out_bf16 else group + 1)) as zwpool,
            tc.tile_pool(name="obf", bufs=obf_bufs) as obfpool,
            tc.tile_pool(name="psum", bufs=4096 // psum_fd, space="PSUM") as psum,
        ):
            xs = inp.tile([2 * DIM, ROWS], BF16)
            ys = inp.tile([2 * DIM, M_COLS], BF16)
            yl = inp.tile([DIM, M_COLS], BF16)
            nc.sync.dma_start(out=xs[:], in_=xs_d[:])
            for q in range(0, M_COLS, 2048):
                nc.sync.dma_start(out=ys[:, q:q + 2048], in_=ys_d[:, q:q + 2048])
                nc.sync.dma_start(out=yl[:, q:q + 2048], in_=yl_d[:, q:q + 2048])

            b200 = consts.tile([128, 1], F32)
            nc.gpsimd.memset(b200[:], 200.0)
            bexp = consts.tile([128, 1], F32)
            nc.gpsimd.memset(bexp[:], float(exp_bias))

            nchunk = M_COLS // psum_fd
            mtile_groups = [
                list(range(g, min(g + group, MTILES)))
                for g in range(0, MTILES, group)
            ]
            loop_cm = tc.For_i(0, iters) if iters > 1 else contextlib.nullcontext(0)
            with loop_cm as _i:
              for grp in mtile_groups:
                  zw_tiles = {}
                  last_evac = None
                  for m in grp:
                      zw = zwpool.tile([128, M_COLS], F32, tag="zw")
                      zw_tiles[m] = zw
                      msl = slice(m * 128, (m + 1) * 128)
                      for nb in range(nchunk):
                          pt = psum.tile([128, psum_fd], F32, tag="ps")
                          for j in range(psum_fd // 512):
                              col = nb * psum_fd + j * 512
                              csl = slice(col, col + 512)
                              nc.tensor.matmul(
                                  pt[:, j * 512:(j + 1) * 512],
                                  xs[:, msl], ys[:, csl],
                                  start=True, stop=False,
                              )
                              nc.tensor.matmul(
                                  pt[:, j * 512:(j + 1) * 512],
                                  xs[:DIM, msl], yl[:, csl],
                                  start=False, stop=True,
                              )
                          sl = slice(nb * psum_fd, (nb + 1) * psum_fd)
                          last_evac = nc.scalar.activation(
                              zw[:, sl], pt[:], AF.Sqrt, bias=b200[:], scale=200.0
                          )
                          nc.vector.scalar_tensor_tensor(
                              zw[:, sl], pt[:], stt_scalar, zw[:, sl],
                              OP.mult, OP.add,
                          )
                  for m in grp:
                      zw = zw_tiles[m]
                      efd = M_COLS // exp_split
                      if out_bf16:
                          obf = obfpool.tile([128, M_COLS], BF16, tag="obf")
                      for e in range(exp_split):
                          esl = slice(e * efd, (e + 1) * efd)
                          etgt = obf[:, esl] if out_bf16 else zw[:, esl]
                          exp_inst = nc.scalar.activation(
                              etgt, zw[:, esl], AF.Exp,
                              bias=bexp[:], scale=float(exp_scale)
                          )
                          if last_evac is not None:
                              add_dep_helper(
                                  exp_inst.ins, last_evac.ins, sync=False,
                                  reason="batch exp after group sqrt (table switch)",
                              )
                          nc.sync.dma_start(
                              out=out_d[m * 128:(m + 1) * 128, esl], in_=etgt
                          )

    nc.finalize()
    return nc


def _build(mode, iters=1, **kw):
    if mode == "gamma":
        return _build_gamma(iters=iters, **kw)
    if mode == "beta2":
        return _build2(iters=iters, **kw)
    return _build_beta(iters=iters, **kw)


LAST_RESULTS = None


def _split_bf16(a):
    hi = a.astype(BFNP)
    lo = (a - hi.astype(np.float32)).astype(BFNP)
    return hi, lo


def kernel(x: np.ndarray, y: np.ndarray) -> np.ndarray:
    global LAST_RESULTS
    x = np.ascontiguousarray(x, dtype=np.float32)
    y = np.ascontiguousarray(y, dtype=np.float32)
    assert x.shape == (N_ROWS, DIM) and y.shape == (M_COLS, DIM)

    if MODE not in _cache:
        _cache[MODE] = _build(MODE)
    nc = _cache[MODE]

    yT = y.T
    yh, yl = _split_bf16(yT)
    ys = np.ascontiguousarray(np.concatenate([yh, yh], axis=0))
    yl = np.ascontiguousarray(yl)

    in_maps = []
    for i in range(N_CORES):
        xT = x[i * ROWS:(i + 1) * ROWS].T
        xh, xl = _split_bf16(xT)
        xstack = np.ascontiguousarray(np.concatenate([xh, xl], axis=0))
        in_maps.append({"xs": xstack, "ys": ys, "yl": yl})

    LAST_RESULTS = run_bass_kernel_spmd(nc, in_maps, list(range(N_CORES)))
    out = np.concatenate([r["out"] for r in LAST_RESULTS.results], axis=0)
    if out.dtype != np.float32:
        out = out.astype(np.float32)
    return out


# revision 7
# speedup vs baseline: 1.3335x; 1.3015x over previous
"""Trainium2 Bass kernel for nn_Bessel: out = i0e(z) * exp(z - 2a), z = 2a*sqrt((1+x@yT)/2), a=10.

Math: out = exp(z - 20 + ln i0e(z)) = exp(A + B*z + C*z^2 - 20) via a minimax
fit of t(z) = z + ln i0e(z) over z in [7.30, 19.20] (max rel err ~5.0e-3).

Key identity used by mode "beta2": the quadratic correction needs no extra
PSUM read since  B*z + C*z^2 = C * v  with  v = (z + B/C) * z  computable on
DVE from the SBUF z-tile alone:
  PE:  c = x@yT into PSUM  (bf16x2 split: c = [xh;xl]@[yh;yh] + xh@yl)
  ACT: z = Sqrt(200c+200) evacuating PSUM   (sole PSUM reader -> PE/ACT
       pipeline runs at full ACT rate; DVE off the recycle path)
  DVE: v = (z + B/C) * z  in-place on the z tile (SBUF-only stt)
  ACT: obf = Exp(C*v + (A-20)) -> bf16
  DMA: obf tile -> HBM (bf16 halves write traffic; host upcasts to f32)

ACT-table batching: M-tiles are processed in GROUPs of 4 (all Sqrt evacs,
then all Exps) so each iteration pays 4 table-set loads; order is pinned
with add_dep_helper so walrus cannot interleave sqrt/exp across a group
boundary.

Mode "beta" is the previous structure (stt reads PSUM, f32 out) kept for A/B.
"""

import contextlib

import numpy as np

import concourse.bacc as bacc
import concourse.mybir as mybir
from concourse.tile import TileContext
from concourse.tile_autobufs import add_dep_helper
from concourse.bass_utils import run_bass_kernel_spmd

AF = mybir.ActivationFunctionType
OP = mybir.AluOpType
F32 = mybir.dt.float32
BF16 = mybir.dt.bfloat16
BFNP = mybir.dt.np(BF16)

N_CORES = 8
N_ROWS, M_COLS, DIM = 8192, 8192, 64
ROWS = N_ROWS // N_CORES          # 1024 rows of x per core
MTILES = ROWS // 128              # 8 partition tiles per core

# minimax coefficients for t(z) = z + ln(i0e(z)) on z in [7.30, 19.20]
BETA_A, BETA_B, BETA_C = -1.36067207867, 0.913667220475, 0.00171853078443

# mode "gamma": minimax fit of the whole exponent as p + r*sqrt(a*u + b),
# u = 200c+200 in [50, 368]; max abs err on the exponent 2.12e-3.
# sqrt input affine = (200a)*c + (200a+b); exp input affine = r*z + p.
GAM_P = -22.179313758272478
GAM_R = 0.7814668006400919
GAM_SQ_SCALE = 314.6476142409728          # 200*a
GAM_SQ_BIAS = 325.04327804569425          # 200*a + b

MODE = "gamma"

_cache = {}


def _build_gamma(group=4, psum_fd=2048, obf_bufs=2, zw_extra=0, exp_split=1,
                 iters=1, zw_fp16=False, no_yl=True, mm_fd=512):
    """mode gamma: exponent = p + r*sqrt(a*u+b) -> no DVE pass at all.

    PE -> ACT Sqrt (PSUM evac, input affine does everything) -> ACT Exp ->
    bf16 out DMA.  M-tiles processed in GROUPs per ACT table set.
    """
    nc = bacc.Bacc(None, target_bir_lowering=False)
    xs_d = nc.dram_tensor("xs", [2 * DIM, ROWS], BF16, kind="ExternalInput")
    ys_d = nc.dram_tensor("ys", [2 * DIM, M_COLS], BF16, kind="ExternalInput")
    yl_d = nc.dram_tensor("yl", [DIM, M_COLS], BF16, kind="ExternalInput")
    out_d = nc.dram_tensor("out", [ROWS, M_COLS], BF16, kind="ExternalOutput")

    zw_dt = mybir.dt.float16 if zw_fp16 else F32

    with TileContext(nc) as tc:
        with (
            tc.tile_pool(name="inp", bufs=1) as inp,
            tc.tile_pool(name="consts", bufs=1) as consts,
            tc.tile_pool(name="zw", bufs=group + zw_extra) as zwpool,
            tc.tile_pool(name="obf", bufs=obf_bufs) as obfpool,
            tc.tile_pool(name="psum", bufs=4096 // psum_fd, space="PSUM") as psum,
        ):
            xs = inp.tile([2 * DIM, ROWS], BF16)
            ys = inp.tile([2 * DIM, M_COLS], BF16)
            yl = inp.tile([DIM, M_COLS], BF16)
            nc.sync.dma_start(out=xs[:], in_=xs_d[:])
            for q in range(0, M_COLS, 2048):
                nc.sync.dma_start(out=ys[:, q:q + 2048], in_=ys_d[:, q:q + 2048])
                nc.sync.dma_start(out=yl[:, q:q + 2048], in_=yl_d[:, q:q + 2048])

            bsq = consts.tile([128, 1], F32)
            nc.gpsimd.memset(bsq[:], float(GAM_SQ_BIAS))
            bexp = consts.tile([128, 1], F32)
            nc.gpsimd.memset(bexp[:], float(GAM_P))

            nchunk = M_COLS // psum_fd
            mtile_groups = [
                list(range(g, min(g + group, MTILES)))
                for g in range(0, MTILES, group)
            ]
            loop_cm = tc.For_i(0, iters) if iters > 1 else contextlib.nullcontext(0)
            with loop_cm as _i:
                last_exp = None
                for grp in mtile_groups:
                    zw_tiles = {}
                    last_evac = None
                    for m in grp:
                        zw = zwpool.tile([128, M_COLS], zw_dt, tag="zw")
                        zw_tiles[m] = zw
                        msl = slice(m * 128, (m + 1) * 128)
                        for nb in range(nchunk):
                            pt = psum.tile([128, psum_fd], F32, tag="ps")
                            for j in range(psum_fd // mm_fd):
                                col = nb * psum_fd + j * mm_fd
                                jsl = slice(j * mm_fd, (j + 1) * mm_fd)
                                nc.tensor.matmul(
                                    pt[:, jsl],
                                    xs[:, msl], ys[:, col:col + mm_fd],
                                    start=True, stop=no_yl,
                                )
                                if not no_yl:
                                    nc.tensor.matmul(
                                        pt[:, jsl],
                                        xs[:DIM, msl], yl[:, col:col + mm_fd],
                                        start=False, stop=True,
                                    )
                            sl = slice(nb * psum_fd, (nb + 1) * psum_fd)
                            ev = nc.scalar.activation(
                                zw[:, sl], pt[:], AF.Sqrt,
                                bias=bsq[:], scale=float(GAM_SQ_SCALE)
                            )
                            if last_exp is not None:
                                add_dep_helper(
                                    ev.ins, last_exp.ins, sync=False,
                                    reason="batch sqrt after prev group exp",
                                )
                            last_evac = ev
                    for m in grp:
                        zw = zw_tiles[m]
                        efd = M_COLS // exp_split
                        obf = obfpool.tile([128, M_COLS], BF16, tag="obf")
                        for e in range(exp_split):
                            esl = slice(e * efd, (e + 1) * efd)
                            exp_inst = nc.scalar.activation(
                                obf[:, esl], zw[:, esl], AF.Exp,
                                bias=bexp[:], scale=float(GAM_R)
                            )
                            add_dep_helper(
                                exp_inst.ins, last_evac.ins, sync=False,
                                reason="batch exp after group sqrt (table switch)",
                            )
                            last_exp = exp_inst
                            nc.sync.dma_start(
                                out=out_d[m * 128:(m + 1) * 128, esl],
                                in_=obf[:, esl],
                            )

    nc.finalize()
    return nc


def _build2(group=4, psum_fd=2048, obf_bufs=2, zw_extra=0, exp_split=1,
            iters=1, out_bf16=True):
    """mode beta2: PSUM-decoupled DVE correction + bf16 output."""
    nc = bacc.Bacc(None, target_bir_lowering=False)
    xs_d = nc.dram_tensor("xs", [2 * DIM, ROWS], BF16, kind="ExternalInput")
    ys_d = nc.dram_tensor("ys", [2 * DIM, M_COLS], BF16, kind="ExternalInput")
    yl_d = nc.dram_tensor("yl", [DIM, M_COLS], BF16, kind="ExternalInput")
    out_d = nc.dram_tensor("out", [ROWS, M_COLS], BF16 if out_bf16 else F32,
                           kind="ExternalOutput")

    stt_scalar = BETA_B / BETA_C                   # v = (z + B/C) * z
    exp_scale = BETA_C                             # exp(C*v + A - 20)
    exp_bias = BETA_A - 20.0

    with TileContext(nc) as tc:
        with (
            tc.tile_pool(name="inp", bufs=1) as inp,
            tc.tile_pool(name="consts", bufs=1) as consts,
            tc.tile_pool(name="zw", bufs=group + zw_extra) as zwpool,
            tc.tile_pool(name="obf", bufs=obf_bufs) as obfpool,
            tc.tile_pool(name="psum", bufs=4096 // psum_fd, space="PSUM") as psum,
        ):
            xs = inp.tile([2 * DIM, ROWS], BF16)
            ys = inp.tile([2 * DIM, M_COLS], BF16)
            yl = inp.tile([DIM, M_COLS], BF16)
            nc.sync.dma_start(out=xs[:], in_=xs_d[:])
            for q in range(0, M_COLS, 2048):
                nc.sync.dma_start(out=ys[:, q:q + 2048], in_=ys_d[:, q:q + 2048])
                nc.sync.dma_start(out=yl[:, q:q + 2048], in_=yl_d[:, q:q + 2048])

            b200 = consts.tile([128, 1], F32)
            nc.gpsimd.memset(b200[:], 200.0)
            bexp = consts.tile([128, 1], F32)
            nc.gpsimd.memset(bexp[:], float(exp_bias))

            nchunk = M_COLS // psum_fd
            mtile_groups = [
                list(range(g, min(g + group, MTILES)))
                for g in range(0, MTILES, group)
            ]
            loop_cm = tc.For_i(0, iters) if iters > 1 else contextlib.nullcontext(0)
            with loop_cm as _i:
                last_exp = None
                for grp in mtile_groups:
                    zw_tiles = {}
                    last_evac = None
                    for m in grp:
                        zw = zwpool.tile([128, M_COLS], F32, tag="zw")
                        zw_tiles[m] = zw
                        msl = slice(m * 128, (m + 1) * 128)
                        for nb in range(nchunk):
                            pt = psum.tile([128, psum_fd], F32, tag="ps")
                            # all hi-part matmuls first, then lo: two
                            # stationary loads per chunk instead of eight
                            for j in range(psum_fd // 512):
                                col = nb * psum_fd + j * 512
                                nc.tensor.matmul(
                                    pt[:, j * 512:(j + 1) * 512],
                                    xs[:, msl], ys[:, col:col + 512],
                                    start=True, stop=False,
                                )
                            for j in range(psum_fd // 512):
                                col = nb * psum_fd + j * 512
                                nc.tensor.matmul(
                                    pt[:, j * 512:(j + 1) * 512],
                                    xs[:DIM, msl], yl[:, col:col + 512],
                                    start=False, stop=True,
                                )
                            sl = slice(nb * psum_fd, (nb + 1) * psum_fd)
                            ev = nc.scalar.activation(
                                zw[:, sl], pt[:], AF.Sqrt, bias=b200[:], scale=200.0
                            )
                            if last_exp is not None:
                                # keep this group's sqrts after the previous
                                # group's exps (table-set batching)
                                add_dep_helper(
                                    ev.ins, last_exp.ins, sync=False,
                                    reason="batch sqrt after prev group exp",
                                )
                            last_evac = ev
                            # v = (z + B/C) * z  -- SBUF-only, PSUM already free
                            nc.vector.scalar_tensor_tensor(
                                zw[:, sl], zw[:, sl], stt_scalar, zw[:, sl],
                                OP.add, OP.mult,
                            )
                    for m in grp:
                        zw = zw_tiles[m]
                        efd = M_COLS // exp_split
                        obf = obfpool.tile([128, M_COLS], BF16 if out_bf16 else F32,
                                           tag="obf")
                        for e in range(exp_split):
                            esl = slice(e * efd, (e + 1) * efd)
                            exp_inst = nc.scalar.activation(
                                obf[:, esl], zw[:, esl], AF.Exp,
                                bias=bexp[:], scale=float(exp_scale)
                            )
                            add_dep_helper(
                                exp_inst.ins, last_evac.ins, sync=False,
                                reason="batch exp after group sqrt (table switch)",
                            )
                            last_exp = exp_inst
                            nc.sync.dma_start(
                                out=out_d[m * 128:(m + 1) * 128, esl],
                                in_=obf[:, esl],
                            )

    nc.finalize()
    return nc


def _build_beta(group=3, zw_bufs=None, exp_split=2, psum_fd=1024,
                iters=1, out_bf16=False, obf_bufs=2):
    """previous structure: stt reads PSUM, default f32 out (for A/B)."""
    nc = bacc.Bacc(None, target_bir_lowering=False)
    xs_d = nc.dram_tensor("xs", [2 * DIM, ROWS], BF16, kind="ExternalInput")
    ys_d = nc.dram_tensor("ys", [2 * DIM, M_COLS], BF16, kind="ExternalInput")
    yl_d = nc.dram_tensor("yl", [DIM, M_COLS], BF16, kind="ExternalInput")
    out_d = nc.dram_tensor("out", [ROWS, M_COLS], BF16 if out_bf16 else F32,
                           kind="ExternalOutput")

    stt_scalar = 200.0 * BETA_C / BETA_B
    exp_scale = BETA_B
    exp_bias = BETA_A + 200.0 * BETA_C - 20.0

    with TileContext(nc) as tc:
        with (
            tc.tile_pool(name="inp", bufs=1) as inp,
            tc.tile_pool(name="consts", bufs=1) as consts,
            tc.tile_pool(name="zw", bufs=zw_bufs or (group if out_bf16 else group + 1)) as zwpool,
            tc.tile_pool(name="obf", bufs=obf_bufs) as obfpool,
            tc.tile_pool(name="psum", bufs=4096 // psum_fd, space="PSUM") as psum,
        ):
            xs = inp.tile([2 * DIM, ROWS], BF16)
            ys = inp.tile([2 * DIM, M_COLS], BF16)
            yl = inp.tile([DIM, M_COLS], BF16)
            nc.sync.dma_start(out=xs[:], in_=xs_d[:])
            for q in range(0, M_COLS, 2048):
                nc.sync.dma_start(out=ys[:, q:q + 2048], in_=ys_d[:, q:q + 2048])
                nc.sync.dma_start(out=yl[:, q:q + 2048], in_=yl_d[:, q:q + 2048])

            b200 = consts.tile([128, 1], F32)
            nc.gpsimd.memset(b200[:], 200.0)
            bexp = consts.tile([128, 1], F32)
            nc.gpsimd.memset(bexp[:], float(exp_bias))

            nchunk = M_COLS // psum_fd
            mtile_groups = [
                list(range(g, min(g + group, MTILES)))
                for g in range(0, MTILES, group)
            ]
            loop_cm = tc.For_i(0, iters) if iters > 1 else contextlib.nullcontext(0)
            with loop_cm as _i:
              for grp in mtile_groups:
                  zw_tiles = {}
                  last_evac = None
                  for m in grp:
                      zw = zwpool.tile([128, M_COLS], F32, tag="zw")
                      zw_tiles[m] = zw
                      msl = slice(m * 128, (m + 1) * 128)
                      for nb in range(nchunk):
                          pt = psum.tile([128, psum_fd], F32, tag="ps")
                          for j in range(psum_fd // 512):
                              col = nb * psum_fd + j * 512
                              csl = slice(col, col + 512)
                              nc.tensor.matmul(
                                  pt[:, j * 512:(j + 1) * 512],
                                  xs[:, msl], ys[:, csl],
                                  start=True, stop=False,
                              )
                              nc.tensor.matmul(
                                  pt[:, j * 512:(j + 1) * 512],
                                  xs[:DIM, msl], yl[:, csl],
                                  start=False, stop=True,
                              )
                          sl = slice(nb * psum_fd, (nb + 1) * psum_fd)
                          last_evac = nc.scalar.activation(
                              zw[:, sl], pt[:], AF.Sqrt, bias=b200[:], scale=200.0
                          )
                          nc.vector.scalar_tensor_tensor(
                              zw[:, sl], pt[:], stt_scalar, zw[:, sl],
                              OP.mult, OP.add,
                          )
                  for m in grp:
                      zw = zw_tiles[m]
                      efd = M_COLS // exp_split
                      if out_bf16:
                          obf = obfpool.tile([128, M_COLS], BF16, tag="obf")
                      for e in range(exp_split):
                          esl = slice(e * efd, (e + 1) * efd)
                          etgt = obf[:, esl] if out_bf16 else zw[:, esl]
                          exp_inst = nc.scalar.activation(
                              etgt, zw[:, esl], AF.Exp,
                              bias=bexp[:], scale=float(exp_scale)
                          )
                          if last_evac is not None:
                              add_dep_helper(
                                  exp_inst.ins, last_evac.ins, sync=False,
                                  reason="batch exp after group sqrt (table switch)",
                              )
                          nc.sync.dma_start(
                              out=out_d[m * 128:(m + 1) * 128, esl], in_=etgt
                          )

    nc.finalize()
    return nc


def _build(mode, iters=1, **kw):
    if mode == "gamma":
        return _build_gamma(iters=iters, **kw)
    if mode == "beta2":
        return _build2(iters=iters, **kw)
    return _build_beta(iters=iters, **kw)


LAST_RESULTS = None


def _split_bf16(a):
    hi = a.astype(BFNP)
    lo = (a - hi.astype(np.float32)).astype(BFNP)
    return hi, lo


def kernel(x: np.ndarray, y: np.ndarray) -> np.ndarray:
    global LAST_RESULTS
    x = np.ascontiguousarray(x, dtype=np.float32)
    y = np.ascontiguousarray(y, dtype=np.float32)
    assert x.shape == (N_ROWS, DIM) and y.shape == (M_COLS, DIM)

    if MODE not in _cache:
        _cache[MODE] = _build(MODE)
    nc = _cache[MODE]

    yT = y.T
    yh, yl = _split_bf16(yT)
    ys = np.ascontiguousarray(np.concatenate([yh, yh], axis=0))
    yl = np.ascontiguousarray(yl)

    in_maps = []
    for i in range(N_CORES):
        xT = x[i * ROWS:(i + 1) * ROWS].T
        xh, xl = _split_bf16(xT)
        xstack = np.ascontiguousarray(np.concatenate([xh, xl], axis=0))
        in_maps.append({"xs": xstack, "ys": ys, "yl": yl})

    LAST_RESULTS = run_bass_kernel_spmd(nc, in_maps, list(range(N_CORES)))
    out = np.concatenate([r["out"] for r in LAST_RESULTS.results], axis=0)
    if out.dtype != np.float32:
        out = out.astype(np.float32)
    return out


# revision 9
# speedup vs baseline: 1.3961x; 1.0469x over previous
"""Trainium2 Bass kernel for nn_Bessel: out = i0e(z) * exp(z - 2a), z = 2a*sqrt((1+x@yT)/2), a=10.

Mode "gamma" (current): the ENTIRE exponent is a 4-parameter minimax fit
  ln out = p + r*sqrt(a*u + b),  u = 200c + 200,  c = x@yT
(max abs err 2.1e-3 over u in [50, 368] -- better than the quadratic-in-z
fit, and the inner affine comes free via the ACT input transform).  Per core
(row-shard of x, y replicated; out is [1024, 8192]):

  PE:  c into PSUM as [xh;xl] @ [yh;yh]  (bf16 split of x only; dropping the
       x@yl cross term costs ~3e-4 L2.  K=128, one matmul per 512 cols.
       NOTE the TRN2 PE p-state: bursts shorter than 3us run at 1.2 GHz, not
       2.4 -- with the yl matmuls the PE paced the whole evac phase.)
  ACT: zw = Sqrt((200a)*c + (200a+b)) evacuating PSUM -> fp16 zw tile
  ACT: obf = Exp(r*zw + p) -> bf16
  DMA: obf -> HBM (bf16 halves write traffic; host upcasts to f32)

No DVE work at all.  ACT is the bottleneck engine and runs at its floor:
2 passes x 1 elem/lane/cycle @ 1.2 GHz = ~115us busy per core, plus 2
activation-table-set loads per iteration (GROUP=8: all 8 M-tiles' sqrts,
then all exps; zw is fp16 so 8 x [128,8192] tiles fit in SBUF).  Ordering
is pinned with add_dep_helper so walrus cannot interleave sqrt/exp phases.

Measured (8-core SPMD, differential For_i timing): see test.py output.
L2 rel err ~4.6e-3 (gate 2e-2).  Modes "beta"/"beta2" kept for A/B.
"""

import contextlib

import numpy as np

import concourse.bacc as bacc
import concourse.mybir as mybir
from concourse.tile import TileContext
from concourse.tile_autobufs import add_dep_helper
from concourse.bass_utils import run_bass_kernel_spmd

AF = mybir.ActivationFunctionType
OP = mybir.AluOpType
F32 = mybir.dt.float32
BF16 = mybir.dt.bfloat16
BFNP = mybir.dt.np(BF16)

N_CORES = 8
N_ROWS, M_COLS, DIM = 8192, 8192, 64
ROWS = N_ROWS // N_CORES          # 1024 rows of x per core
MTILES = ROWS // 128              # 8 partition tiles per core

# minimax coefficients for t(z) = z + ln(i0e(z)) on z in [7.30, 19.20]
BETA_A, BETA_B, BETA_C = -1.36067207867, 0.913667220475, 0.00171853078443

# mode "gamma": minimax fit of the whole exponent as p + r*sqrt(a*u + b),
# u = 200c+200 in [50, 368]; max abs err on the exponent 2.12e-3.
# sqrt input affine = (200a)*c + (200a+b); exp input affine = r*z + p.
GAM_P = -22.179313758272478
GAM_R = 0.7814668006400919
GAM_SQ_SCALE = 314.6476142409728          # 200*a
GAM_SQ_BIAS = 325.04327804569425          # 200*a + b

MODE = "gamma"

_cache = {}


def _build_gamma(group=8, psum_fd=2048, obf_bufs=3, zw_extra=0, exp_split=1,
                 iters=1, zw_fp16=True, no_yl=True, mm_fd=512):
    """mode gamma: exponent = p + r*sqrt(a*u+b) -> no DVE pass at all.

    PE -> ACT Sqrt (PSUM evac, input affine does everything) -> ACT Exp ->
    bf16 out DMA.  M-tiles processed in GROUPs per ACT table set.
    """
    nc = bacc.Bacc(None, target_bir_lowering=False)
    xs_d = nc.dram_tensor("xs", [2 * DIM, ROWS], BF16, kind="ExternalInput")
    ys_d = nc.dram_tensor("ys", [2 * DIM, M_COLS], BF16, kind="ExternalInput")
    yl_d = None if no_yl else nc.dram_tensor("yl", [DIM, M_COLS], BF16, kind="ExternalInput")
    out_d = nc.dram_tensor("out", [ROWS, M_COLS], BF16, kind="ExternalOutput")

    zw_dt = mybir.dt.float16 if zw_fp16 else F32

    with TileContext(nc) as tc:
        with (
            tc.tile_pool(name="inp", bufs=1) as inp,
            tc.tile_pool(name="consts", bufs=1) as consts,
            tc.tile_pool(name="zw", bufs=group + zw_extra) as zwpool,
            tc.tile_pool(name="obf", bufs=obf_bufs) as obfpool,
            tc.tile_pool(name="psum", bufs=4096 // psum_fd, space="PSUM") as psum,
        ):
            xs = inp.tile([2 * DIM, ROWS], BF16)
            ys = inp.tile([2 * DIM, M_COLS], BF16)
            yl = None if no_yl else inp.tile([DIM, M_COLS], BF16)
            nc.sync.dma_start(out=xs[:], in_=xs_d[:])
            for q in range(0, M_COLS, 2048):
                nc.sync.dma_start(out=ys[:, q:q + 2048], in_=ys_d[:, q:q + 2048])
                if not no_yl:
                    nc.sync.dma_start(out=yl[:, q:q + 2048], in_=yl_d[:, q:q + 2048])

            bsq = consts.tile([128, 1], F32)
            nc.gpsimd.memset(bsq[:], float(GAM_SQ_BIAS))
            bexp = consts.tile([128, 1], F32)
            nc.gpsimd.memset(bexp[:], float(GAM_P))

            nchunk = M_COLS // psum_fd
            mtile_groups = [
                list(range(g, min(g + group, MTILES)))
                for g in range(0, MTILES, group)
            ]
            loop_cm = tc.For_i(0, iters) if iters > 1 else contextlib.nullcontext(0)
            with loop_cm as _i:
                last_exp = None
                for grp in mtile_groups:
                    zw_tiles = {}
                    last_evac = None
                    for m in grp:
                        zw = zwpool.tile([128, M_COLS], zw_dt, tag="zw")
                        zw_tiles[m] = zw
                        msl = slice(m * 128, (m + 1) * 128)
                        for nb in range(nchunk):
                            pt = psum.tile([128, psum_fd], F32, tag="ps")
                            for j in range(psum_fd // mm_fd):
                                col = nb * psum_fd + j * mm_fd
                                jsl = slice(j * mm_fd, (j + 1) * mm_fd)
                                nc.tensor.matmul(
                                    pt[:, jsl],
                                    xs[:, msl], ys[:, col:col + mm_fd],
                                    start=True, stop=no_yl,
                                )
                                if not no_yl:
                                    nc.tensor.matmul(
                                        pt[:, jsl],
                                        xs[:DIM, msl], yl[:, col:col + mm_fd],
                                        start=False, stop=True,
                                    )
                            sl = slice(nb * psum_fd, (nb + 1) * psum_fd)
                            ev = nc.scalar.activation(
                                zw[:, sl], pt[:], AF.Sqrt,
                                bias=bsq[:], scale=float(GAM_SQ_SCALE)
                            )
                            if last_exp is not None:
                                add_dep_helper(
                                    ev.ins, last_exp.ins, sync=False,
                                    reason="batch sqrt after prev group exp",
                                )
                            last_evac = ev
                    for m in grp:
                        zw = zw_tiles[m]
                        efd = M_COLS // exp_split
                        obf = obfpool.tile([128, M_COLS], BF16, tag="obf")
                        for e in range(exp_split):
                            esl = slice(e * efd, (e + 1) * efd)
                            exp_inst = nc.scalar.activation(
                                obf[:, esl], zw[:, esl], AF.Exp,
                                bias=bexp[:], scale=float(GAM_R)
                            )
                            add_dep_helper(
                                exp_inst.ins, last_evac.ins, sync=False,
                                reason="batch exp after group sqrt (table switch)",
                            )
                            last_exp = exp_inst
                            nc.sync.dma_start(
                                out=out_d[m * 128:(m + 1) * 128, esl],
                                in_=obf[:, esl],
                            )

    nc.finalize()
    return nc


def _build2(group=4, psum_fd=2048, obf_bufs=2, zw_extra=0, exp_split=1,
            iters=1, out_bf16=True):
    """mode beta2: PSUM-decoupled DVE correction + bf16 output."""
    nc = bacc.Bacc(None, target_bir_lowering=False)
    xs_d = nc.dram_tensor("xs", [2 * DIM, ROWS], BF16, kind="ExternalInput")
    ys_d = nc.dram_tensor("ys", [2 * DIM, M_COLS], BF16, kind="ExternalInput")
    yl_d = nc.dram_tensor("yl", [DIM, M_COLS], BF16, kind="ExternalInput")
    out_d = nc.dram_tensor("out", [ROWS, M_COLS], BF16 if out_bf16 else F32,
                           kind="ExternalOutput")

    stt_scalar = BETA_B / BETA_C                   # v = (z + B/C) * z
    exp_scale = BETA_C                             # exp(C*v + A - 20)
    exp_bias = BETA_A - 20.0

    with TileContext(nc) as tc:
        with (
            tc.tile_pool(name="inp", bufs=1) as inp,
            tc.tile_pool(name="consts", bufs=1) as consts,
            tc.tile_pool(name="zw", bufs=group + zw_extra) as zwpool,
            tc.tile_pool(name="obf", bufs=obf_bufs) as obfpool,
            tc.tile_pool(name="psum", bufs=4096 // psum_fd, space="PSUM") as psum,
        ):
            xs = inp.tile([2 * DIM, ROWS], BF16)
            ys = inp.tile([2 * DIM, M_COLS], BF16)
            yl = inp.tile([DIM, M_COLS], BF16)
            nc.sync.dma_start(out=xs[:], in_=xs_d[:])
            for q in range(0, M_COLS, 2048):
                nc.sync.dma_start(out=ys[:, q:q + 2048], in_=ys_d[:, q:q + 2048])
                nc.sync.dma_start(out=yl[:, q:q + 2048], in_=yl_d[:, q:q + 2048])

            b200 = consts.tile([128, 1], F32)
            nc.gpsimd.memset(b200[:], 200.0)
            bexp = consts.tile([128, 1], F32)
            nc.gpsimd.memset(bexp[:], float(exp_bias))

            nchunk = M_COLS // psum_fd
            mtile_groups = [
                list(range(g, min(g + group, MTILES)))
                for g in range(0, MTILES, group)
            ]
            loop_cm = tc.For_i(0, iters) if iters > 1 else contextlib.nullcontext(0)
            with loop_cm as _i:
                last_exp = None
                for grp in mtile_groups:
                    zw_tiles = {}
                    last_evac = None
                    for m in grp:
                        zw = zwpool.tile([128, M_COLS], F32, tag="zw")
                        zw_tiles[m] = zw
                        msl = slice(m * 128, (m + 1) * 128)
                        for nb in range(nchunk):
                            pt = psum.tile([128, psum_fd], F32, tag="ps")
                            # all hi-part matmuls first, then lo: two
                            # stationary loads per chunk instead of eight
                            for j in range(psum_fd // 512):
                                col = nb * psum_fd + j * 512
                                nc.tensor.matmul(
                                    pt[:, j * 512:(j + 1) * 512],
                                    xs[:, msl], ys[:, col:col + 512],
                                    start=True, stop=False,
                                )
                            for j in range(psum_fd // 512):
                                col = nb * psum_fd + j * 512
                                nc.tensor.matmul(
                                    pt[:, j * 512:(j + 1) * 512],
                                    xs[:DIM, msl], yl[:, col:col + 512],
                                    start=False, stop=True,
                                )
                            sl = slice(nb * psum_fd, (nb + 1) * psum_fd)
                            ev = nc.scalar.activation(
                                zw[:, sl], pt[:], AF.Sqrt, bias=b200[:], scale=200.0
                            )
                            if last_exp is not None:
                                # keep this group's sqrts after the previous
                                # group's exps (table-set batching)
                                add_dep_helper(
                                    ev.ins, last_exp.ins, sync=False,
                                    reason="batch sqrt after prev group exp",
                                )
                            last_evac = ev
                            # v = (z + B/C) * z  -- SBUF-only, PSUM already free
                            nc.vector.scalar_tensor_tensor(
                                zw[:, sl], zw[:, sl], stt_scalar, zw[:, sl],
                                OP.add, OP.mult,
                            )
                    for m in grp:
                        zw = zw_tiles[m]
                        efd = M_COLS // exp_split
                        obf = obfpool.tile([128, M_COLS], BF16 if out_bf16 else F32,
                                           tag="obf")
                        for e in range(exp_split):
                            esl = slice(e * efd, (e + 1) * efd)
                            exp_inst = nc.scalar.activation(
                                obf[:, esl], zw[:, esl], AF.Exp,
                                bias=bexp[:], scale=float(exp_scale)
                            )
                            add_dep_helper(
                                exp_inst.ins, last_evac.ins, sync=False,
                                reason="batch exp after group sqrt (table switch)",
                            )
                            last_exp = exp_inst
                            nc.sync.dma_start(
                                out=out_d[m * 128:(m + 1) * 128, esl],
                                in_=obf[:, esl],
                            )

    nc.finalize()
    return nc


def _build_beta(group=3, zw_bufs=None, exp_split=2, psum_fd=1024,
                iters=1, out_bf16=False, obf_bufs=2):
    """previous structure: stt reads PSUM, default f32 out (for A/B)."""
    nc = bacc.Bacc(None, target_bir_lowering=False)
    xs_d = nc.dram_tensor("xs", [2 * DIM, ROWS], BF16, kind="ExternalInput")
    ys_d = nc.dram_tensor("ys", [2 * DIM, M_COLS], BF16, kind="ExternalInput")
    yl_d = nc.dram_tensor("yl", [DIM, M_COLS], BF16, kind="ExternalInput")
    out_d = nc.dram_tensor("out", [ROWS, M_COLS], BF16 if out_bf16 else F32,
                           kind="ExternalOutput")

    stt_scalar = 200.0 * BETA_C / BETA_B
    exp_scale = BETA_B
    exp_bias = BETA_A + 200.0 * BETA_C - 20.0

    with TileContext(nc) as tc:
        with (
            tc.tile_pool(name="inp", bufs=1) as inp,
            tc.tile_pool(name="consts", bufs=1) as consts,
            tc.tile_pool(name="zw", bufs=zw_bufs or (group if out_bf16 else group + 1)) as zwpool,
            tc.tile_pool(name="obf", bufs=obf_bufs) as obfpool,
            tc.tile_pool(name="psum", bufs=4096 // psum_fd, space="PSUM") as psum,
        ):
            xs = inp.tile([2 * DIM, ROWS], BF16)
            ys = inp.tile([2 * DIM, M_COLS], BF16)
            yl = inp.tile([DIM, M_COLS], BF16)
            nc.sync.dma_start(out=xs[:], in_=xs_d[:])
            for q in range(0, M_COLS, 2048):
                nc.sync.dma_start(out=ys[:, q:q + 2048], in_=ys_d[:, q:q + 2048])
                nc.sync.dma_start(out=yl[:, q:q + 2048], in_=yl_d[:, q:q + 2048])

            b200 = consts.tile([128, 1], F32)
            nc.gpsimd.memset(b200[:], 200.0)
            bexp = consts.tile([128, 1], F32)
            nc.gpsimd.memset(bexp[:], float(exp_bias))

            nchunk = M_COLS // psum_fd
            mtile_groups = [
                list(range(g, min(g + group, MTILES)))
                for g in range(0, MTILES, group)
            ]
            loop_cm = tc.For_i(0, iters) if iters > 1 else contextlib.nullcontext(0)
            with loop_cm as _i:
              for grp in mtile_groups:
                  zw_tiles = {}
                  last_evac = None
                  for m in grp:
                      zw = zwpool.tile([128, M_COLS], F32, tag="zw")
                      zw_tiles[m] = zw
                      msl = slice(m * 128, (m + 1) * 128)
                      for nb in range(nchunk):
                          pt = psum.tile([128, psum_fd], F32, tag="ps")
                          for j in range(psum_fd // 512):
                              col = nb * psum_fd + j * 512
                              csl = slice(col, col + 512)
                              nc.tensor.matmul(
                                  pt[:, j * 512:(j + 1) * 512],
                                  xs[:, msl], ys[:, csl],
                                  start=True, stop=False,
                              )
                              nc.tensor.matmul(
                                  pt[:, j * 512:(j + 1) * 512],
                                  xs[:DIM, msl], yl[:, csl],
                                  start=False, stop=True,
                              )
                          sl = slice(nb * psum_fd, (nb + 1) * psum_fd)
                          last_evac = nc.scalar.activation(
                              zw[:, sl], pt[:], AF.Sqrt, bias=b200[:], scale=200.0
                          )
                          nc.vector.scalar_tensor_tensor(
                              zw[:, sl], pt[:], stt_scalar, zw[:, sl],
                              OP.mult, OP.add,
                          )
                  for m in grp:
                      zw = zw_tiles[m]
                      efd = M_COLS // exp_split
                      if out_bf16:
                          obf = obfpool.tile([128, M_COLS], BF16, tag="obf")
                      for e in range(exp_split):
                          esl = slice(e * efd, (e + 1) * efd)
                          etgt = obf[:, esl] if out_bf16 else zw[:, esl]
                          exp_inst = nc.scalar.activation(
                              etgt, zw[:, esl], AF.Exp,
                              bias=bexp[:], scale=float(exp_scale)
                          )
                          if last_evac is not None:
                              add_dep_helper(
                                  exp_inst.ins, last_evac.ins, sync=False,
                                  reason="batch exp after group sqrt (table switch)",
                              )
                          nc.sync.dma_start(
                              out=out_d[m * 128:(m + 1) * 128, esl], in_=etgt
                          )

    nc.finalize()
    return nc


def _build(mode, iters=1, **kw):
    if mode == "gamma":
        return _build_gamma(iters=iters, **kw)
    if mode == "beta2":
        return _build2(iters=iters, **kw)
    return _build_beta(iters=iters, **kw)


LAST_RESULTS = None


def _split_bf16(a):
    hi = a.astype(BFNP)
    lo = (a - hi.astype(np.float32)).astype(BFNP)
    return hi, lo


def make_in_maps(x, y, with_yl=None):
    if with_yl is None:
        with_yl = MODE != "gamma"
    yT = y.T
    yh, yl = _split_bf16(yT)
    ys = np.ascontiguousarray(np.concatenate([yh, yh], axis=0))
    yl = np.ascontiguousarray(yl)
    in_maps = []
    for i in range(N_CORES):
        xT = x[i * ROWS:(i + 1) * ROWS].T
        xh, xl = _split_bf16(xT)
        xstack = np.ascontiguousarray(np.concatenate([xh, xl], axis=0))
        m = {"xs": xstack, "ys": ys}
        if with_yl:
            m["yl"] = yl
        in_maps.append(m)
    return in_maps


def kernel(x: np.ndarray, y: np.ndarray) -> np.ndarray:
    global LAST_RESULTS
    x = np.ascontiguousarray(x, dtype=np.float32)
    y = np.ascontiguousarray(y, dtype=np.float32)
    assert x.shape == (N_ROWS, DIM) and y.shape == (M_COLS, DIM)

    if MODE not in _cache:
        _cache[MODE] = _build(MODE)
    nc = _cache[MODE]

    in_maps = make_in_maps(x, y)

    LAST_RESULTS = run_bass_kernel_spmd(nc, in_maps, list(range(N_CORES)))
    out = np.concatenate([r["out"] for r in LAST_RESULTS.results], axis=0)
    if out.dtype != np.float32:
        out = out.astype(np.float32)
    return out


# revision 10
# speedup vs baseline: 1.4141x; 1.0129x over previous
"""Trainium2 Bass kernel for nn_Bessel: out = i0e(z) * exp(z - 2a), z = 2a*sqrt((1+x@yT)/2), a=10.

Mode "gamma" (current): the ENTIRE exponent is a 4-parameter minimax fit
  ln out = p + r*sqrt(a*u + b),  u = 200c + 200,  c = x@yT
(max abs err 2.1e-3 over u in [50, 368] -- better than the quadratic-in-z
fit, and the inner affine comes free via the ACT input transform).  Per core
(row-shard of x, y replicated; out is [1024, 8192]):

  PE:  c into PSUM as [xh;xl] @ [yh;yh]  (bf16 split of x only; dropping the
       x@yl cross term costs ~3e-4 L2.  K=128, one matmul per 512 cols.
       NOTE the TRN2 PE p-state: bursts shorter than 3us run at 1.2 GHz, not
       2.4 -- with the yl matmuls the PE paced the whole evac phase.)
  ACT: zw = Sqrt((200a)*c + (200a+b)) evacuating PSUM -> fp16 zw tile
  ACT: obf = Exp(r*zw + p) -> bf16
  DMA: obf -> HBM (bf16 halves write traffic; host upcasts to f32)

No DVE work at all.  ACT is the bottleneck engine and runs at its floor:
2 passes x 1 elem/lane/cycle @ 1.2 GHz = ~115us busy per core, plus 2
activation-table-set loads per iteration (GROUP=8: all 8 M-tiles' sqrts,
then all exps; zw is fp16 so 8 x [128,8192] tiles fit in SBUF).  Ordering
is pinned with add_dep_helper so walrus cannot interleave sqrt/exp phases.

Measured (8-core SPMD, differential For_i timing): see test.py output.
L2 rel err ~4.6e-3 (gate 2e-2).  Modes "beta"/"beta2" kept for A/B.
"""

import contextlib

import numpy as np

import concourse.bacc as bacc
import concourse.mybir as mybir
from concourse.tile import TileContext
from concourse.tile_autobufs import add_dep_helper
from concourse.bass_utils import run_bass_kernel_spmd

AF = mybir.ActivationFunctionType
OP = mybir.AluOpType
F32 = mybir.dt.float32
BF16 = mybir.dt.bfloat16
BFNP = mybir.dt.np(BF16)

N_CORES = 8
N_ROWS, M_COLS, DIM = 8192, 8192, 64
ROWS = N_ROWS // N_CORES          # 1024 rows of x per core
MTILES = ROWS // 128              # 8 partition tiles per core

# minimax coefficients for t(z) = z + ln(i0e(z)) on z in [7.30, 19.20]
BETA_A, BETA_B, BETA_C = -1.36067207867, 0.913667220475, 0.00171853078443

# mode "gamma": minimax fit of the whole exponent as p + r*sqrt(a*u + b),
# u = 200c+200 in [50, 368]; max abs err on the exponent 2.12e-3.
# sqrt input affine = (200a)*c + (200a+b); exp input affine = r*z + p.
GAM_P = -22.179313758272478
GAM_R = 0.7814668006400919
GAM_SQ_SCALE = 314.6476142409728          # 200*a
GAM_SQ_BIAS = 325.04327804569425          # 200*a + b

MODE = "gamma"

_cache = {}


def _build_gamma(group=8, psum_fd=1024, obf_bufs=3, zw_extra=0, exp_split=1,
                 iters=1, zw_fp16=True, no_yl=True, mm_fd=512):
    """mode gamma: exponent = p + r*sqrt(a*u+b) -> no DVE pass at all.

    PE -> ACT Sqrt (PSUM evac, input affine does everything) -> ACT Exp ->
    bf16 out DMA.  M-tiles processed in GROUPs per ACT table set.
    """
    nc = bacc.Bacc(None, target_bir_lowering=False)
    xs_d = nc.dram_tensor("xs", [2 * DIM, ROWS], BF16, kind="ExternalInput")
    ys_d = nc.dram_tensor("ys", [2 * DIM, M_COLS], BF16, kind="ExternalInput")
    yl_d = None if no_yl else nc.dram_tensor("yl", [DIM, M_COLS], BF16, kind="ExternalInput")
    out_d = nc.dram_tensor("out", [ROWS, M_COLS], BF16, kind="ExternalOutput")

    zw_dt = mybir.dt.float16 if zw_fp16 else F32

    with TileContext(nc) as tc:
        with (
            tc.tile_pool(name="inp", bufs=1) as inp,
            tc.tile_pool(name="consts", bufs=1) as consts,
            tc.tile_pool(name="zw", bufs=group + zw_extra) as zwpool,
            tc.tile_pool(name="obf", bufs=obf_bufs) as obfpool,
            tc.tile_pool(name="psum", bufs=4096 // psum_fd, space="PSUM") as psum,
        ):
            xs = inp.tile([2 * DIM, ROWS], BF16)
            ys = inp.tile([2 * DIM, M_COLS], BF16)
            yl = None if no_yl else inp.tile([DIM, M_COLS], BF16)
            nc.sync.dma_start(out=xs[:], in_=xs_d[:])
            for q in range(0, M_COLS, 2048):
                nc.sync.dma_start(out=ys[:, q:q + 2048], in_=ys_d[:, q:q + 2048])
                if not no_yl:
                    nc.sync.dma_start(out=yl[:, q:q + 2048], in_=yl_d[:, q:q + 2048])

            bsq = consts.tile([128, 1], F32)
            nc.gpsimd.memset(bsq[:], float(GAM_SQ_BIAS))
            bexp = consts.tile([128, 1], F32)
            nc.gpsimd.memset(bexp[:], float(GAM_P))

            nchunk = M_COLS // psum_fd
            mtile_groups = [
                list(range(g, min(g + group, MTILES)))
                for g in range(0, MTILES, group)
            ]
            loop_cm = tc.For_i(0, iters) if iters > 1 else contextlib.nullcontext(0)
            with loop_cm as _i:
                last_exp = None
                for grp in mtile_groups:
                    zw_tiles = {}
                    last_evac = None
                    for m in grp:
                        zw = zwpool.tile([128, M_COLS], zw_dt, tag="zw")
                        zw_tiles[m] = zw
                        msl = slice(m * 128, (m + 1) * 128)
                        for nb in range(nchunk):
                            pt = psum.tile([128, psum_fd], F32, tag="ps")
                            for j in range(psum_fd // mm_fd):
                                col = nb * psum_fd + j * mm_fd
                                jsl = slice(j * mm_fd, (j + 1) * mm_fd)
                                nc.tensor.matmul(
                                    pt[:, jsl],
                                    xs[:, msl], ys[:, col:col + mm_fd],
                                    start=True, stop=no_yl,
                                )
                                if not no_yl:
                                    nc.tensor.matmul(
                                        pt[:, jsl],
                                        xs[:DIM, msl], yl[:, col:col + mm_fd],
                                        start=False, stop=True,
                                    )
                            sl = slice(nb * psum_fd, (nb + 1) * psum_fd)
                            ev = nc.scalar.activation(
                                zw[:, sl], pt[:], AF.Sqrt,
                                bias=bsq[:], scale=float(GAM_SQ_SCALE)
                            )
                            if last_exp is not None:
                                add_dep_helper(
                                    ev.ins, last_exp.ins, sync=False,
                                    reason="batch sqrt after prev group exp",
                                )
                            last_evac = ev
                    for m in grp:
                        zw = zw_tiles[m]
                        efd = M_COLS // exp_split
                        obf = obfpool.tile([128, M_COLS], BF16, tag="obf")
                        for e in range(exp_split):
                            esl = slice(e * efd, (e + 1) * efd)
                            exp_inst = nc.scalar.activation(
                                obf[:, esl], zw[:, esl], AF.Exp,
                                bias=bexp[:], scale=float(GAM_R)
                            )
                            add_dep_helper(
                                exp_inst.ins, last_evac.ins, sync=False,
                                reason="batch exp after group sqrt (table switch)",
                            )
                            last_exp = exp_inst
                            nc.sync.dma_start(
                                out=out_d[m * 128:(m + 1) * 128, esl],
                                in_=obf[:, esl],
                            )

    nc.finalize()
    return nc


def _build2(group=4, psum_fd=2048, obf_bufs=2, zw_extra=0, exp_split=1,
            iters=1, out_bf16=True):
    """mode beta2: PSUM-decoupled DVE correction + bf16 output."""
    nc = bacc.Bacc(None, target_bir_lowering=False)
    xs_d = nc.dram_tensor("xs", [2 * DIM, ROWS], BF16, kind="ExternalInput")
    ys_d = nc.dram_tensor("ys", [2 * DIM, M_COLS], BF16, kind="ExternalInput")
    yl_d = nc.dram_tensor("yl", [DIM, M_COLS], BF16, kind="ExternalInput")
    out_d = nc.dram_tensor("out", [ROWS, M_COLS], BF16 if out_bf16 else F32,
                           kind="ExternalOutput")

    stt_scalar = BETA_B / BETA_C                   # v = (z + B/C) * z
    exp_scale = BETA_C                             # exp(C*v + A - 20)
    exp_bias = BETA_A - 20.0

    with TileContext(nc) as tc:
        with (
            tc.tile_pool(name="inp", bufs=1) as inp,
            tc.tile_pool(name="consts", bufs=1) as consts,
            tc.tile_pool(name="zw", bufs=group + zw_extra) as zwpool,
            tc.tile_pool(name="obf", bufs=obf_bufs) as obfpool,
            tc.tile_pool(name="psum", bufs=4096 // psum_fd, space="PSUM") as psum,
        ):
            xs = inp.tile([2 * DIM, ROWS], BF16)
            ys = inp.tile([2 * DIM, M_COLS], BF16)
            yl = inp.tile([DIM, M_COLS], BF16)
            nc.sync.dma_start(out=xs[:], in_=xs_d[:])
            for q in range(0, M_COLS, 2048):
                nc.sync.dma_start(out=ys[:, q:q + 2048], in_=ys_d[:, q:q + 2048])
                nc.sync.dma_start(out=yl[:, q:q + 2048], in_=yl_d[:, q:q + 2048])

            b200 = consts.tile([128, 1], F32)
            nc.gpsimd.memset(b200[:], 200.0)
            bexp = consts.tile([128, 1], F32)
            nc.gpsimd.memset(bexp[:], float(exp_bias))

            nchunk = M_COLS // psum_fd
            mtile_groups = [
                list(range(g, min(g + group, MTILES)))
                for g in range(0, MTILES, group)
            ]
            loop_cm = tc.For_i(0, iters) if iters > 1 else contextlib.nullcontext(0)
            with loop_cm as _i:
                last_exp = None
                for grp in mtile_groups:
                    zw_tiles = {}
                    last_evac = None
                    for m in grp:
                        zw = zwpool.tile([128, M_COLS], F32, tag="zw")
                        zw_tiles[m] = zw
                        msl = slice(m * 128, (m + 1) * 128)
                        for nb in range(nchunk):
                            pt = psum.tile([128, psum_fd], F32, tag="ps")
                            # all hi-part matmuls first, then lo: two
                            # stationary loads per chunk instead of eight
                            for j in range(psum_fd // 512):
                                col = nb * psum_fd + j * 512
                                nc.tensor.matmul(
                                    pt[:, j * 512:(j + 1) * 512],
                                    xs[:, msl], ys[:, col:col + 512],
                                    start=True, stop=False,
                                )
                            for j in range(psum_fd // 512):
                                col = nb * psum_fd + j * 512
                                nc.tensor.matmul(
                                    pt[:, j * 512:(j + 1) * 512],
                                    xs[:DIM, msl], yl[:, col:col + 512],
                                    start=False, stop=True,
                                )
                            sl = slice(nb * psum_fd, (nb + 1) * psum_fd)
                            ev = nc.scalar.activation(
                                zw[:, sl], pt[:], AF.Sqrt, bias=b200[:], scale=200.0
                            )
                            if last_exp is not None:
                                # keep this group's sqrts after the previous
                                # group's exps (table-set batching)
                                add_dep_helper(
                                    ev.ins, last_exp.ins, sync=False,
                                    reason="batch sqrt after prev group exp",
                                )
                            last_evac = ev
                            # v = (z + B/C) * z  -- SBUF-only, PSUM already free
                            nc.vector.scalar_tensor_tensor(
                                zw[:, sl], zw[:, sl], stt_scalar, zw[:, sl],
                                OP.add, OP.mult,
                            )
                    for m in grp:
                        zw = zw_tiles[m]
                        efd = M_COLS // exp_split
                        obf = obfpool.tile([128, M_COLS], BF16 if out_bf16 else F32,
                                           tag="obf")
                        for e in range(exp_split):
                            esl = slice(e * efd, (e + 1) * efd)
                            exp_inst = nc.scalar.activation(
                                obf[:, esl], zw[:, esl], AF.Exp,
                                bias=bexp[:], scale=float(exp_scale)
                            )
                            add_dep_helper(
                                exp_inst.ins, last_evac.ins, sync=False,
                                reason="batch exp after group sqrt (table switch)",
                            )
                            last_exp = exp_inst
                            nc.sync.dma_start(
                                out=out_d[m * 128:(m + 1) * 128, esl],
                                in_=obf[:, esl],
                            )

    nc.finalize()
    return nc


def _build_beta(group=3, zw_bufs=None, exp_split=2, psum_fd=1024,
                iters=1, out_bf16=False, obf_bufs=2):
    """previous structure: stt reads PSUM, default f32 out (for A/B)."""
    nc = bacc.Bacc(None, target_bir_lowering=False)
    xs_d = nc.dram_tensor("xs", [2 * DIM, ROWS], BF16, kind="ExternalInput")
    ys_d = nc.dram_tensor("ys", [2 * DIM, M_COLS], BF16, kind="ExternalInput")
    yl_d = nc.dram_tensor("yl", [DIM, M_COLS], BF16, kind="ExternalInput")
    out_d = nc.dram_tensor("out", [ROWS, M_COLS], BF16 if out_bf16 else F32,
                           kind="ExternalOutput")

    stt_scalar = 200.0 * BETA_C / BETA_B
    exp_scale = BETA_B
    exp_bias = BETA_A + 200.0 * BETA_C - 20.0

    with TileContext(nc) as tc:
        with (
            tc.tile_pool(name="inp", bufs=1) as inp,
            tc.tile_pool(name="consts", bufs=1) as consts,
            tc.tile_pool(name="zw", bufs=zw_bufs or (group if out_bf16 else group + 1)) as zwpool,
            tc.tile_pool(name="obf", bufs=obf_bufs) as obfpool,
            tc.tile_pool(name="psum", bufs=4096 // psum_fd, space="PSUM") as psum,
        ):
            xs = inp.tile([2 * DIM, ROWS], BF16)
            ys = inp.tile([2 * DIM, M_COLS], BF16)
            yl = inp.tile([DIM, M_COLS], BF16)
            nc.sync.dma_start(out=xs[:], in_=xs_d[:])
            for q in range(0, M_COLS, 2048):
                nc.sync.dma_start(out=ys[:, q:q + 2048], in_=ys_d[:, q:q + 2048])
                nc.sync.dma_start(out=yl[:, q:q + 2048], in_=yl_d[:, q:q + 2048])

            b200 = consts.tile([128, 1], F32)
            nc.gpsimd.memset(b200[:], 200.0)
            bexp = consts.tile([128, 1], F32)
            nc.gpsimd.memset(bexp[:], float(exp_bias))

            nchunk = M_COLS // psum_fd
            mtile_groups = [
                list(range(g, min(g + group, MTILES)))
                for g in range(0, MTILES, group)
            ]
            loop_cm = tc.For_i(0, iters) if iters > 1 else contextlib.nullcontext(0)
            with loop_cm as _i:
              for grp in mtile_groups:
                  zw_tiles = {}
                  last_evac = None
                  for m in grp:
                      zw = zwpool.tile([128, M_COLS], F32, tag="zw")
                      zw_tiles[m] = zw
                      msl = slice(m * 128, (m + 1) * 128)
                      for nb in range(nchunk):
                          pt = psum.tile([128, psum_fd], F32, tag="ps")
                          for j in range(psum_fd // 512):
                              col = nb * psum_fd + j * 512
                              csl = slice(col, col + 512)
                              nc.tensor.matmul(
                                  pt[:, j * 512:(j + 1) * 512],
                                  xs[:, msl], ys[:, csl],
                                  start=True, stop=False,
                              )
                              nc.tensor.matmul(
                                  pt[:, j * 512:(j + 1) * 512],
                                  xs[:DIM, msl], yl[:, csl],
                                  start=False, stop=True,
                              )
                          sl = slice(nb * psum_fd, (nb + 1) * psum_fd)
                          last_evac = nc.scalar.activation(
                              zw[:, sl], pt[:], AF.Sqrt, bias=b200[:], scale=200.0
                          )
                          nc.vector.scalar_tensor_tensor(
                              zw[:, sl], pt[:], stt_scalar, zw[:, sl],
                              OP.mult, OP.add,
                          )
                  for m in grp:
                      zw = zw_tiles[m]
                      efd = M_COLS // exp_split
                      if out_bf16:
                          obf = obfpool.tile([128, M_COLS], BF16, tag="obf")
                      for e in range(exp_split):
                          esl = slice(e * efd, (e + 1) * efd)
                          etgt = obf[:, esl] if out_bf16 else zw[:, esl]
                          exp_inst = nc.scalar.activation(
                              etgt, zw[:, esl], AF.Exp,
                              bias=bexp[:], scale=float(exp_scale)
                          )
                          if last_evac is not None:
                              add_dep_helper(
                                  exp_inst.ins, last_evac.ins, sync=False,
                                  reason="batch exp after group sqrt (table switch)",
                              )
                          nc.sync.dma_start(
                              out=out_d[m * 128:(m + 1) * 128, esl], in_=etgt
                          )

    nc.finalize()
    return nc


def _build(mode, iters=1, **kw):
    if mode == "gamma":
        return _build_gamma(iters=iters, **kw)
    if mode == "beta2":
        return _build2(iters=iters, **kw)
    return _build_beta(iters=iters, **kw)


LAST_RESULTS = None


def _split_bf16(a):
    hi = a.astype(BFNP)
    lo = (a - hi.astype(np.float32)).astype(BFNP)
    return hi, lo


def make_in_maps(x, y, with_yl=None):
    if with_yl is None:
        with_yl = MODE != "gamma"
    yT = y.T
    yh, yl = _split_bf16(yT)
    ys = np.ascontiguousarray(np.concatenate([yh, yh], axis=0))
    yl = np.ascontiguousarray(yl)
    in_maps = []
    for i in range(N_CORES):
        xT = x[i * ROWS:(i + 1) * ROWS].T
        xh, xl = _split_bf16(xT)
        xstack = np.ascontiguousarray(np.concatenate([xh, xl], axis=0))
        m = {"xs": xstack, "ys": ys}
        if with_yl:
            m["yl"] = yl
        in_maps.append(m)
    return in_maps


def kernel(x: np.ndarray, y: np.ndarray) -> np.ndarray:
    global LAST_RESULTS
    x = np.ascontiguousarray(x, dtype=np.float32)
    y = np.ascontiguousarray(y, dtype=np.float32)
    assert x.shape == (N_ROWS, DIM) and y.shape == (M_COLS, DIM)

    if MODE not in _cache:
        _cache[MODE] = _build(MODE)
    nc = _cache[MODE]

    in_maps = make_in_maps(x, y)

    LAST_RESULTS = run_bass_kernel_spmd(nc, in_maps, list(range(N_CORES)))
    out = np.concatenate([r["out"] for r in LAST_RESULTS.results], axis=0)
    if out.dtype != np.float32:
        out = out.astype(np.float32)
    return out
